# revision 4
# baseline (speedup 1.0000x reference)
"""Trainium2 Bass kernel for nn_Clustering_80900003987951 (vq_codebook).

Math (reference):
  x: [B=128, S=128, F=64, 1], centroids: [1, K=64, S=128, F=64]
  d2[b,k,s] = sum_f (x[b,s,f] - c[k,s,f])^2
  dist[b,k] = sum_s sqrt(d2[b,k,s])
  q = (1 + dist^2/2)^-3, normalized over k                  -> [B, K]

Strategy: shard the SEQUENCE dim across the 8 cores (S_loc=16), keep the
full batch on every core. Per-core input drops to ~400KB (vs 1.36MB for
batch sharding, where every core must load all centroids), matmuls use
all 128 output partitions, and the device returns the per-core partial
  qp[b,k] = sum_{s in shard} sqrt(d2[b,k,s])     [128, 64] f32
The host sums the 8 partials and applies the tiny q tail (25K flops,
~0.002% of the work) exactly in float64.

Device pipeline per core:
  xt [66, S_loc*B]: rows 0-63 = x^T (F on partitions), 64 = 1, 65 = |x|^2
  ct [66, S_loc*K]: rows 0-63 = -2*c^T, 64 = |c|^2, 65 = 1
  per s: d2 tile = xt_s^T @ ct_s  -> PSUM [128, 64]   (16 matmuls, 2 banks)
  ACT sqrt per bank -> fp16 sbuf; one DVE reduce over s; DMA out.
Inputs stream on 4 parallel DGE queues (sync/vector/scalar/gpsimd) so the
transfer isn't serialized behind a single queue like the old version.
"""

import numpy as np

B, K, S, F = 128, 64, 128, 64
NCORES = 8
SLOC = S // NCORES          # 16 sequence positions per core
HALF = SLOC // 2            # 8 per PSUM bank
CP = F + 2                  # 66 contraction rows (data + aug)

# fp16 operands: halves DMA bytes and avoids the 2-pass fp32 PE matmul;
# fp32 PSUM accumulation keeps the error ~1e-4.
XT_DT = "float16"
CT_DT = "float16"
DI_DT = "float16"           # sqrt results; fp16 doubles DVE reduce speed

_CACHE = {}


def _build_nc():
    import concourse.bacc as bacc
    import concourse.tile as tile
    from concourse import mybir

    f32 = mybir.dt.float32
    fxt = getattr(mybir.dt, XT_DT)
    fct = getattr(mybir.dt, CT_DT)
    fdi = getattr(mybir.dt, DI_DT)
    nc = bacc.Bacc("TRN2", target_bir_lowering=False, debug=False)

    xt0_d = nc.dram_tensor("xt0", [CP, HALF * B], fxt, kind="ExternalInput")
    xt1_d = nc.dram_tensor("xt1", [CP, HALF * B], fxt, kind="ExternalInput")
    ct0_d = nc.dram_tensor("ct0", [CP, HALF * K], fct, kind="ExternalInput")
    ct1_d = nc.dram_tensor("ct1", [CP, HALF * K], fct, kind="ExternalInput")
    qp_d = nc.dram_tensor("qp", [B, K], f32, kind="ExternalOutput")

    with tile.TileContext(nc) as tc:
        with (
            tc.tile_pool(name="ins", bufs=1) as in_pool,
            tc.tile_pool(name="psum", bufs=1, space="PSUM") as psum_pool,
            tc.tile_pool(name="work", bufs=1) as work_pool,
        ):
            xt0_t = in_pool.tile([CP, HALF * B], fxt, name="xt0t")
            xt1_t = in_pool.tile([CP, HALF * B], fxt, name="xt1t")
            ct0_t = in_pool.tile([CP, HALF * K], fct, name="ct0t")
            ct1_t = in_pool.tile([CP, HALF * K], fct, name="ct1t")
            # 2 parallel DGE queues (only SP/Activation issue HWDGE DMAs;
            # gpsimd SWDGE crashed the exec unit). Bank-0 operands first.
            nc.sync.dma_start(out=xt0_t[:], in_=xt0_d.ap())
            nc.scalar.dma_start(out=ct0_t[:], in_=ct0_d.ap())
            nc.sync.dma_start(out=xt1_t[:], in_=xt1_d.ap())
            nc.scalar.dma_start(out=ct1_t[:], in_=ct1_d.ap())

            ps0 = psum_pool.tile([128, HALF * K], f32, name="ps0")
            ps1 = psum_pool.tile([128, HALF * K], f32, name="ps1")
            di = work_pool.tile([128, SLOC, K], fdi, name="di")

            for u in range(HALF):
                nc.tensor.matmul(
                    ps0[:, u * K:(u + 1) * K],
                    lhsT=xt0_t[:, u * B:(u + 1) * B],
                    rhs=ct0_t[:, u * K:(u + 1) * K],
                    start=True,
                    stop=True,
                )
            for u in range(HALF):
                nc.tensor.matmul(
                    ps1[:, u * K:(u + 1) * K],
                    lhsT=xt1_t[:, u * B:(u + 1) * B],
                    rhs=ct1_t[:, u * K:(u + 1) * K],
                    start=True,
                    stop=True,
                )

            nc.scalar.activation(
                di[:, 0:HALF, :], ps0[:], mybir.ActivationFunctionType.Sqrt
            )
            nc.scalar.activation(
                di[:, HALF:SLOC, :], ps1[:], mybir.ActivationFunctionType.Sqrt
            )

            qt = work_pool.tile([B, K], f32, name="qt")
            nc.vector.tensor_reduce(
                qt[:],
                di[:].rearrange("p t k -> p k t"),
                axis=mybir.AxisListType.X,
                op=mybir.AluOpType.add,
            )
            nc.sync.dma_start(out=qp_d.ap(), in_=qt[:])

    nc.compile()
    return nc


def _prep_inputs(x, centroids):
    """Host-side shard + transpose + augmentation. Returns in_maps list."""
    from concourse import mybir

    xt_np = mybir.dt.np(getattr(mybir.dt, XT_DT))
    ct_np = mybir.dt.np(getattr(mybir.dt, CT_DT))
    x = np.ascontiguousarray(np.asarray(x, dtype=np.float32)).reshape(B, S, F)
    c = np.ascontiguousarray(np.asarray(centroids, dtype=np.float32)).reshape(K, S, F)

    in_maps = []
    for i in range(NCORES):
        m = {}
        for h, name in ((0, "xt0"), (1, "xt1")):
            sl = slice(i * SLOC + h * HALF, i * SLOC + (h + 1) * HALF)
            xs = x[:, sl, :]                              # [B, HALF, F]
            xt = np.empty((CP, HALF * B), dtype=xt_np)
            xt[:F] = xs.transpose(2, 1, 0).reshape(F, HALF * B)
            xt[F] = 1.0
            xt[F + 1] = ((xs * xs).sum(-1, dtype=np.float32).T).reshape(HALF * B)
            m[name] = xt
        for h, name in ((0, "ct0"), (1, "ct1")):
            sl = slice(i * SLOC + h * HALF, i * SLOC + (h + 1) * HALF)
            cs = c[:, sl, :]                              # [K, HALF, F]
            ct = np.empty((CP, HALF * K), dtype=ct_np)
            ct[:F] = (-2.0 * cs).transpose(2, 1, 0).reshape(F, HALF * K)
            ct[F] = ((cs * cs).sum(-1, dtype=np.float32).T).reshape(HALF * K)
            ct[F + 1] = 1.0
            m[name] = ct
        in_maps.append(m)
    return in_maps


def kernel(x, centroids):
    from concourse.bass_utils import run_bass_kernel_spmd

    if "nc" not in _CACHE:
        _CACHE["nc"] = _build_nc()
    nc = _CACHE["nc"]

    in_maps = _prep_inputs(x, centroids)
    res = run_bass_kernel_spmd(nc, in_maps, core_ids=list(range(NCORES)))
    dist = np.zeros((B, K), dtype=np.float64)
    for i in range(NCORES):
        dist += res.results[i]["qp"].astype(np.float64)
    # q tail (exact, host): q = (1 + d^2/2)^-3 normalized over k
    q = 1.0 / (1.0 + dist * dist / 2.0)
    q = q * q * q
    q = q / q.sum(axis=1, keepdims=True)
    return q.astype(np.float32)
